# revision 5
# baseline (speedup 1.0000x reference)
"""GCNAggregator Trainium2 Bass kernel.

out[i] = (sum_{e: seg[e]==i} features[neighbor_idx[e]] + features[i]) / (deg_i + 1)

Strategy (8 NeuronCores, SPMD):
  - Nodes are sharded into 8 contiguous ranges of 6272 (=49*128) node slots.
    Since segment_ids is sorted, each core's incident edges are a contiguous
    range of the edge list.
  - Per core, nodes are processed in 49 groups of 128 nodes. Each group's
    edges are gathered from HBM with gpsimd.dma_gather (1KB rows) and
    segment-summed on the tensor engine via one-hot matmuls:
        psum[128 nodes, 256] += onehot[128 edges, 128 nodes]^T @ G[128 edges, 256]
    The one-hot is built on device from per-edge relative segment ids
    (is_equal against an iota row).
  - dma_gather indices are int16 (max 32767) but the table has 50000 rows,
    so each group's edges are split into a low class (row < 32768) and a
    high class (row >= 32768, gathered from an offset view of the table).
  - A single dma_gather is limited to 1024 indices (SWDGE descriptor ring
    capacity), so each class run is issued as <=1024-row gather calls into
    column slices of one per-group SBUF tile.
  - Finalize per group: out = (psum + features[self]) * 1/(deg+1), DMA out.

The host only computes integer index metadata (shard boundaries, per-group
class-split index streams, relative segment ids, degrees); all floating
point work (gather, segment sum, self-add, normalize) runs on device.
"""

import sys

import numpy as np

try:
    import concourse  # noqa: F401
except ImportError:  # pragma: no cover
    sys.path.insert(0, "/opt/trn_rl_repo")

from contextlib import ExitStack

import concourse.mybir as mybir
from concourse import bacc, bass_utils, tile

N_NODES = 50000
N_EDGES = 1_600_000
D = 256
N_CORES = 8
NPC = 6272          # node slots per core (= GROUPS * 128)
GROUPS = 49
SPLIT = 32768       # int16 gather-index window
H_ROWS = N_NODES - SPLIT

_PROGRAM_CACHE: dict = {}
LAST_NC = None  # exposed for test harness introspection (TimelineSim)

MAX_GATHER = 1024  # SWDGE ring capacity per dma_gather call


def _chunks(total_rows):
    out = []
    off = 0
    while off < total_rows:
        k = min(MAX_GATHER, total_rows - off)
        out.append((off, k))
        off += k
    return out


def _build_program(t_l: int, t_h: int):
    """Build + compile the (uniform, SPMD) per-core Bass program."""
    tiles_per_group = t_l + t_h
    nt_tot = GROUPS * tiles_per_group
    rows_tot = nt_tot * 128
    ni16 = rows_tot // 16  # gidx columns (wrapped-16 int16 layout)

    nc = bacc.Bacc(
        "TRN2", target_bir_lowering=False, debug=False, num_devices=N_CORES
    )

    feat_d = nc.dram_tensor(
        "features", (N_NODES, D), mybir.dt.float32, kind="ExternalInput"
    ).ap()
    gidx_d = nc.dram_tensor(
        "gidx", (128, ni16), mybir.dt.int16, kind="ExternalInput"
    ).ap()
    srel_d = nc.dram_tensor(
        "srel", (128, nt_tot), mybir.dt.float32, kind="ExternalInput"
    ).ap()
    cnt1_d = nc.dram_tensor(
        "cnt1", (128, GROUPS), mybir.dt.float32, kind="ExternalInput"
    ).ap()
    fself_d = nc.dram_tensor(
        "fself", (NPC, D), mybir.dt.float32, kind="ExternalInput"
    ).ap()
    out_d = nc.dram_tensor(
        "out", (NPC, D), mybir.dt.float32, kind="ExternalOutput"
    ).ap()

    feat_lo = feat_d[0:SPLIT, :]
    feat_hi = feat_d[SPLIT:N_NODES, :]

    with tile.TileContext(nc) as tc:
        with ExitStack() as ctx:
            const_pool = ctx.enter_context(tc.tile_pool(name="const", bufs=1))
            g_pool = ctx.enter_context(tc.tile_pool(name="gt", bufs=2))
            oh_pool = ctx.enter_context(tc.tile_pool(name="oh", bufs=4))
            fin_pool = ctx.enter_context(tc.tile_pool(name="fin", bufs=3))
            psum_pool = ctx.enter_context(
                tc.tile_pool(name="psum", bufs=2, space="PSUM")
            )

            # persistent metadata in SBUF
            gidx_sb = const_pool.tile([128, ni16], mybir.dt.int16)
            nc.sync.dma_start(gidx_sb[:], gidx_d[:])
            srel_sb = const_pool.tile([128, nt_tot], mybir.dt.float32)
            nc.sync.dma_start(srel_sb[:], srel_d[:])
            cnt1_sb = const_pool.tile([128, GROUPS], mybir.dt.float32)
            nc.sync.dma_start(cnt1_sb[:], cnt1_d[:])

            iota_i = const_pool.tile([128, 128], mybir.dt.int32)
            nc.gpsimd.iota(iota_i[:], pattern=[[1, 128]], base=0, channel_multiplier=0)
            iota_f = const_pool.tile([128, 128], mybir.dt.float32)
            nc.vector.tensor_copy(iota_f[:], iota_i[:])

            kl = t_l * 128
            kh = t_h * 128
            icols = tiles_per_group * 8  # gidx cols per group (128 rows -> 8 cols)

            for g in range(GROUPS):
                c0 = g * icols
                gt = g_pool.tile(
                    [128, tiles_per_group, D], mybir.dt.float32, tag="gt"
                )
                for off, k in _chunks(kl):
                    nc.gpsimd.dma_gather(
                        gt[:, off // 128 : (off + k) // 128, :], feat_lo,
                        gidx_sb[:, c0 + off // 16 : c0 + (off + k) // 16],
                        num_idxs=k, num_idxs_reg=k, elem_size=D, elem_step=D,
                    )
                for off, k in _chunks(kh):
                    nc.gpsimd.dma_gather(
                        gt[:, t_l + off // 128 : t_l + (off + k) // 128, :],
                        feat_hi,
                        gidx_sb[
                            :,
                            c0 + t_l * 8 + off // 16 : c0 + t_l * 8 + (off + k) // 16,
                        ],
                        num_idxs=k, num_idxs_reg=k, elem_size=D, elem_step=D,
                    )

                psum = psum_pool.tile([128, D], mybir.dt.float32, tag="ps")
                m0 = g * tiles_per_group
                for t in range(tiles_per_group):
                    oh = oh_pool.tile([128, 128], mybir.dt.float32, tag="oh")
                    nc.vector.tensor_scalar(
                        oh[:], iota_f[:], srel_sb[:, m0 + t : m0 + t + 1], None,
                        op0=mybir.AluOpType.is_equal,
                    )
                    nc.tensor.matmul(
                        psum[:], oh[:], gt[:, t, :],
                        start=(t == 0), stop=(t == tiles_per_group - 1),
                    )

                fs = fin_pool.tile([128, D], mybir.dt.float32, tag="fs")
                nc.sync.dma_start(fs[:], fself_d[g * 128 : (g + 1) * 128, :])
                rec = fin_pool.tile([128, 1], mybir.dt.float32, tag="rec")
                nc.vector.reciprocal(rec[:], cnt1_sb[:, g : g + 1])
                o_sb = fin_pool.tile([128, D], mybir.dt.float32, tag="o")
                nc.vector.tensor_add(o_sb[:], psum[:], fs[:])
                nc.vector.tensor_scalar_mul(o_sb[:], o_sb[:], rec[:])
                nc.sync.dma_start(out_d[g * 128 : (g + 1) * 128, :], o_sb[:])

    nc.compile()
    return nc


def _preprocess(features, neighbor_idx, segment_ids):
    """Host-side shard/index metadata construction (integers only)."""
    feat = np.ascontiguousarray(np.asarray(features, dtype=np.float32))
    seg = np.asarray(segment_ids).astype(np.int64)
    nid = np.asarray(neighbor_idx).astype(np.int64)

    deg = np.bincount(seg, minlength=N_CORES * NPC)
    core_bounds = np.searchsorted(
        seg, np.clip(np.arange(N_CORES + 1) * NPC, 0, N_NODES)
    )

    # first pass: per (core, group, class) edge counts -> uniform tile counts
    per_core = []
    max_l = 0
    max_h = 0
    for c in range(N_CORES):
        lo, hi = core_bounds[c], core_bounds[c + 1]
        s = seg[lo:hi] - c * NPC
        x = nid[lo:hi]
        gb = np.searchsorted(s, np.arange(GROUPS + 1) * 128)
        per_core.append((s, x, gb))
        is_l = x < SPLIT
        cl = np.bincount((s >> 7)[is_l], minlength=GROUPS)
        ch = np.bincount((s >> 7)[~is_l], minlength=GROUPS)
        if len(cl):
            max_l = max(max_l, int(cl.max()))
            max_h = max(max_h, int(ch.max()))
    t_l = max(1, -(-max_l // 128))
    t_h = max(1, -(-max_h // 128))

    tiles_per_group = t_l + t_h
    nt_tot = GROUPS * tiles_per_group
    kl, kh = t_l * 128, t_h * 128

    in_maps = []
    for c in range(N_CORES):
        s, x, gb = per_core[c]
        gidx_all = np.zeros(nt_tot * 128, np.int16)
        srel_all = np.full((nt_tot, 128), -1.0, np.float32)
        for g in range(GROUPS):
            a, b = gb[g], gb[g + 1]
            sg = s[a:b]
            xg = x[a:b]
            m = xg < SPLIT
            xl = xg[m]
            xh = xg[~m] - SPLIT
            sl = sg[m] - g * 128
            sh = sg[~m] - g * 128
            base = g * tiles_per_group * 128
            gidx_all[base : base + len(xl)] = xl.astype(np.int16)
            gidx_all[base + kl : base + kl + len(xh)] = xh.astype(np.int16)
            m0 = g * tiles_per_group
            srl = np.full(kl, -1.0, np.float32)
            srl[: len(sl)] = sl
            srel_all[m0 : m0 + t_l] = srl.reshape(t_l, 128)
            srh = np.full(kh, -1.0, np.float32)
            srh[: len(sh)] = sh
            srel_all[m0 + t_l : m0 + tiles_per_group] = srh.reshape(t_h, 128)

        gidx_w = np.ascontiguousarray(
            np.tile(gidx_all.reshape(-1, 16).T, (8, 1))
        )
        srel_mat = np.ascontiguousarray(srel_all.T)
        cnt1 = np.ascontiguousarray(
            1.0 + deg[c * NPC : (c + 1) * NPC].reshape(GROUPS, 128).T
        ).astype(np.float32)
        fself = np.zeros((NPC, D), np.float32)
        n_real = min(NPC, N_NODES - c * NPC)
        fself[:n_real] = feat[c * NPC : c * NPC + n_real]
        in_maps.append(
            {
                "features": feat,
                "gidx": gidx_w,
                "srel": srel_mat,
                "cnt1": cnt1,
                "fself": fself,
            }
        )
    return t_l, t_h, in_maps


def kernel(features, neighbor_idx, segment_ids):
    global LAST_NC
    t_l, t_h, in_maps = _preprocess(features, neighbor_idx, segment_ids)

    key = (t_l, t_h)
    if key not in _PROGRAM_CACHE:
        _PROGRAM_CACHE[key] = _build_program(t_l, t_h)
    nc = _PROGRAM_CACHE[key]
    LAST_NC = nc

    res = bass_utils.run_bass_kernel_spmd(
        nc, in_maps, core_ids=list(range(N_CORES))
    )

    out = np.empty((N_NODES, D), np.float32)
    for c in range(N_CORES):
        lo = c * NPC
        n_real = min(NPC, N_NODES - lo)
        out[lo : lo + n_real] = res.results[c]["out"][:n_real]
    return out


# revision 16
# speedup vs baseline: 1.0773x; 1.0773x over previous
"""GCNAggregator Trainium2 Bass kernel.

out[i] = (sum_{e: seg[e]==i} features[neighbor_idx[e]] + features[i]) / (deg_i + 1)

Strategy (8 NeuronCores, SPMD):
  - Nodes are sharded into 8 contiguous ranges of 6272 (=49*128) node slots.
    Since segment_ids is sorted, each core's incident edges are a contiguous
    range of the edge list.
  - Per core, nodes are processed in 49 groups of 128 nodes. Each group's
    edges are gathered from HBM with gpsimd.dma_gather (1KB rows) and
    segment-summed on the tensor engine via one-hot matmuls:
        psum[128 nodes, 256] += onehot[128 edges, 128 nodes]^T @ G[128 edges, 256]
    The one-hot is built on device from per-edge relative segment ids
    (is_equal against an iota row).
  - dma_gather indices are int16 (max 32767) but the table has 50000 rows,
    so each group's edges are split into a low class (row < 32768) and a
    high class (row >= 32768, gathered from an offset view of the table).
  - A single dma_gather is limited to 1024 indices (SWDGE descriptor ring
    capacity), so each class run is issued as <=1024-row gather calls into
    column slices of one per-group SBUF tile.
  - Finalize per group: out = (psum + features[self]) * 1/(deg+1), DMA out.

The host only computes integer index metadata (shard boundaries, per-group
class-split index streams, relative segment ids, degrees); all floating
point work (gather, segment sum, self-add, normalize) runs on device.
"""

import sys

import numpy as np

try:
    import concourse  # noqa: F401
except ImportError:  # pragma: no cover
    sys.path.insert(0, "/opt/trn_rl_repo")

from contextlib import ExitStack

import concourse.mybir as mybir
from concourse import bacc, bass_utils, tile

N_NODES = 50000
N_EDGES = 1_600_000
D = 256
N_CORES = 8
NPC = 6272          # node slots per core (= GROUPS * 128)
GROUPS = 49
SPLIT = 32768       # int16 gather-index window
H_ROWS = N_NODES - SPLIT

_PROGRAM_CACHE: dict = {}
LAST_NC = None  # exposed for test harness introspection (TimelineSim)

MAX_GATHER = 1024  # SWDGE ring capacity per dma_gather call


def _chunks(total_rows):
    out = []
    off = 0
    while off < total_rows:
        k = min(MAX_GATHER, total_rows - off)
        out.append((off, k))
        off += k
    return out


def _build_program(t_l_arr: tuple, t_h_arr: tuple):
    """Build + compile the (uniform-across-cores, SPMD) per-core program.

    t_l_arr/t_h_arr: per-group tile counts (max over the 8 cores), so the
    program structure is identical on every core while padding stays low.
    """
    n_slots = len(t_l_arr)
    tiles_g = [t_l_arr[g] + t_h_arr[g] for g in range(n_slots)]
    nt_tot = sum(tiles_g)
    rows_tot = nt_tot * 128
    ni16 = rows_tot // 16  # gidx columns (wrapped-16 int16 layout)
    # column offset of each group's tile block
    m_off = np.concatenate([[0], np.cumsum(tiles_g)]).astype(int)

    nc = bacc.Bacc(
        "TRN2", target_bir_lowering=False, debug=False, num_devices=N_CORES
    )

    feat_d = nc.dram_tensor(
        "features", (N_NODES, D), mybir.dt.float32, kind="ExternalInput"
    ).ap()
    gidx_d = nc.dram_tensor(
        "gidx", (128, ni16), mybir.dt.int16, kind="ExternalInput"
    ).ap()
    srel_d = nc.dram_tensor(
        "srel", (128, nt_tot), mybir.dt.float32, kind="ExternalInput"
    ).ap()
    cnt1_d = nc.dram_tensor(
        "cnt1", (128, n_slots), mybir.dt.float32, kind="ExternalInput"
    ).ap()
    fself_d = nc.dram_tensor(
        "fself", (n_slots * 128, D), mybir.dt.float32, kind="ExternalInput"
    ).ap()
    out_d = nc.dram_tensor(
        "out", (n_slots * 128, D), mybir.dt.float32, kind="ExternalOutput"
    ).ap()

    feat_lo = feat_d[0:SPLIT, :]
    feat_hi = feat_d[SPLIT:N_NODES, :]

    with tile.TileContext(nc) as tc:
        with ExitStack() as ctx:
            import os

            gb = int(os.environ.get("GT_BUFS", "2"))
            ob = int(os.environ.get("OH_BUFS", "4"))
            fb = int(os.environ.get("FIN_BUFS", "3"))
            pb = int(os.environ.get("PSUM_BUFS", "2"))
            const_pool = ctx.enter_context(tc.tile_pool(name="const", bufs=1))
            g_pool = ctx.enter_context(tc.tile_pool(name="gt", bufs=gb))
            oh_pool = ctx.enter_context(tc.tile_pool(name="oh", bufs=ob))
            fin_pool = ctx.enter_context(tc.tile_pool(name="fin", bufs=fb))
            psum_pool = ctx.enter_context(
                tc.tile_pool(name="psum", bufs=pb, space="PSUM")
            )

            # persistent metadata in SBUF (gidx loaded in chunks so early
            # gathers don't wait on the full 3.4MB index transfer)
            gidx_sb = const_pool.tile([128, ni16], mybir.dt.int16)
            n_ld = 8
            ld_bounds = [ni16 * i // n_ld for i in range(n_ld + 1)]
            for a, b in zip(ld_bounds[:-1], ld_bounds[1:]):
                if b > a:
                    nc.sync.dma_start(gidx_sb[:, a:b], gidx_d[:, a:b])
            srel_sb = const_pool.tile([128, nt_tot], mybir.dt.float32)
            nc.sync.dma_start(srel_sb[:], srel_d[:])
            cnt1_sb = const_pool.tile([128, n_slots], mybir.dt.float32)
            nc.sync.dma_start(cnt1_sb[:], cnt1_d[:])

            iota_i = const_pool.tile([128, 128], mybir.dt.int32)
            nc.gpsimd.iota(iota_i[:], pattern=[[1, 128]], base=0, channel_multiplier=0)
            iota_f = const_pool.tile([128, 128], mybir.dt.float32)
            nc.vector.tensor_copy(iota_f[:], iota_i[:])

            max_tiles = max(tiles_g)
            for g in range(n_slots):
                t_l = t_l_arr[g]
                n_tiles = tiles_g[g]
                m0 = int(m_off[g])
                c0 = m0 * 8  # 128 rows -> 8 int16-wrapped columns
                gt = g_pool.tile([128, max_tiles, D], mybir.dt.float32, tag="gt")
                for off, k in _chunks(t_l * 128):
                    nc.gpsimd.dma_gather(
                        gt[:, off // 128 : (off + k) // 128, :], feat_lo,
                        gidx_sb[:, c0 + off // 16 : c0 + (off + k) // 16],
                        num_idxs=k, num_idxs_reg=k, elem_size=D, elem_step=D,
                    )
                for off, k in _chunks(t_h_arr[g] * 128):
                    nc.gpsimd.dma_gather(
                        gt[:, t_l + off // 128 : t_l + (off + k) // 128, :],
                        feat_hi,
                        gidx_sb[
                            :,
                            c0 + t_l * 8 + off // 16 : c0 + t_l * 8 + (off + k) // 16,
                        ],
                        num_idxs=k, num_idxs_reg=k, elem_size=D, elem_step=D,
                    )

                psum = psum_pool.tile([128, D], mybir.dt.float32, tag="ps")
                for t in range(n_tiles):
                    oh = oh_pool.tile([128, 128], mybir.dt.float32, tag="oh")
                    nc.vector.tensor_scalar(
                        oh[:], iota_f[:], srel_sb[:, m0 + t : m0 + t + 1], None,
                        op0=mybir.AluOpType.is_equal,
                    )
                    nc.tensor.matmul(
                        psum[:], oh[:], gt[:, t, :],
                        start=(t == 0), stop=(t == n_tiles - 1),
                    )

                fs = fin_pool.tile([128, D], mybir.dt.float32, tag="fs")
                nc.sync.dma_start(fs[:], fself_d[g * 128 : (g + 1) * 128, :])
                rec = fin_pool.tile([128, 1], mybir.dt.float32, tag="rec")
                nc.vector.reciprocal(rec[:], cnt1_sb[:, g : g + 1])
                o_sb = fin_pool.tile([128, D], mybir.dt.float32, tag="o")
                nc.vector.tensor_add(o_sb[:], psum[:], fs[:])
                nc.vector.tensor_scalar_mul(o_sb[:], o_sb[:], rec[:])
                nc.sync.dma_start(out_d[g * 128 : (g + 1) * 128, :], o_sb[:])

    nc.compile()
    return nc


def _pack_slots(cum_l, cum_h, n_nodes, cap_l, cap_h):
    """Greedy variable-width node slots: each slot takes consecutive nodes
    (<=128) while its L/H edge counts stay under the caps. Returns a list of
    (base, width, nL, nH)."""
    slots = []
    i = 0
    while i < n_nodes:
        jmax = min(i + 128, n_nodes)
        jl = int(np.searchsorted(cum_l, cum_l[i] + cap_l * 128, side="right")) - 1
        jh = int(np.searchsorted(cum_h, cum_h[i] + cap_h * 128, side="right")) - 1
        j = max(min(jmax, jl, jh), i + 1)
        slots.append(
            (i, j - i, int(cum_l[j] - cum_l[i]), int(cum_h[j] - cum_h[i]))
        )
        i = j
    return slots


def _preprocess(features, neighbor_idx, segment_ids):
    """Host-side shard/index metadata construction (integers only)."""
    feat = np.ascontiguousarray(np.asarray(features, dtype=np.float32))
    seg = np.asarray(segment_ids).astype(np.int64)
    nid = np.asarray(neighbor_idx).astype(np.int64)
    n_edges = seg.shape[0]

    deg = np.bincount(seg, minlength=N_NODES)

    # edge-balanced core node boundaries (spans capped at NPC node slots)
    bounds = [0]
    for c in range(1, N_CORES):
        n = int(seg[min(c * n_edges // N_CORES, n_edges - 1)])
        n = min(n, bounds[-1] + NPC)
        n = max(n, N_NODES - (N_CORES - c) * NPC, bounds[-1])
        bounds.append(n)
    bounds.append(N_NODES)

    # per-core edge slices and per-node class-split prefix sums
    per_core = []
    for c in range(N_CORES):
        lo, hi = np.searchsorted(seg, [bounds[c], bounds[c + 1]])
        s = seg[lo:hi] - bounds[c]
        x = nid[lo:hi]
        nn = bounds[c + 1] - bounds[c]
        is_l = x < SPLIT
        cnt_l = np.bincount(s[is_l], minlength=nn)
        cnt_h = np.bincount(s[~is_l], minlength=nn)
        cum_l = np.concatenate([[0], np.cumsum(cnt_l)])
        cum_h = np.concatenate([[0], np.cumsum(cnt_h)])
        per_core.append((s, x, nn, cum_l, cum_h))

    # choose caps minimizing total (uniform-across-cores) tile count
    best = None
    for cap_l in range(17, 24):
        for cap_h in range(9, 13):
            all_slots = [
                _pack_slots(pc[3], pc[4], pc[2], cap_l, cap_h) for pc in per_core
            ]
            n_slots = max(len(sl) for sl in all_slots)
            tl = np.zeros(n_slots, np.int64)
            th = np.zeros(n_slots, np.int64)
            for sl in all_slots:
                for g, (_, _, nl, nh) in enumerate(sl):
                    tl[g] = max(tl[g], -(-nl // 128))
                    th[g] = max(th[g], -(-nh // 128))
            total = int(tl.sum() + th.sum())
            if best is None or total < best[0]:
                best = (total, tuple(int(v) for v in tl), tuple(int(v) for v in th), all_slots)
    _, t_l_arr, t_h_arr, all_slots = best
    # a slot with zero tiles would leave its PSUM accumulator unwritten
    t_l_arr = tuple(
        max(tl, 1) if tl + th == 0 else tl for tl, th in zip(t_l_arr, t_h_arr)
    )
    n_slots = len(t_l_arr)

    tiles_g = [t_l_arr[g] + t_h_arr[g] for g in range(n_slots)]
    nt_tot = sum(tiles_g)
    m_off = np.concatenate([[0], np.cumsum(tiles_g)]).astype(int)

    in_maps = []
    slot_maps = []
    for c in range(N_CORES):
        s, x, nn, _, _ = per_core[c]
        slots = all_slots[c]
        gidx_all = np.zeros(nt_tot * 128, np.int16)
        srel_all = np.full((nt_tot, 128), -1.0, np.float32)
        cnt1 = np.ones((128, n_slots), np.float32)
        fself = np.zeros((n_slots * 128, D), np.float32)
        node_bnds = [sl[0] for sl in slots] + [nn]
        edge_bnds = np.searchsorted(s, node_bnds)
        for g, (base_n, width, _, _) in enumerate(slots):
            t_l, t_h = t_l_arr[g], t_h_arr[g]
            kl, kh = t_l * 128, t_h * 128
            a, b = edge_bnds[g], edge_bnds[g + 1]
            sg = s[a:b]
            xg = x[a:b]
            m = xg < SPLIT
            xl = xg[m]
            xh = xg[~m] - SPLIT
            sl_ = sg[m] - base_n
            sh_ = sg[~m] - base_n
            m0 = int(m_off[g])
            base = m0 * 128
            gidx_all[base : base + len(xl)] = xl.astype(np.int16)
            gidx_all[base + kl : base + kl + len(xh)] = xh.astype(np.int16)
            srl = np.full(kl, -1.0, np.float32)
            srl[: len(sl_)] = sl_
            srel_all[m0 : m0 + t_l] = srl.reshape(t_l, 128)
            srh = np.full(kh, -1.0, np.float32)
            srh[: len(sh_)] = sh_
            srel_all[m0 + t_l : m0 + t_l + t_h] = srh.reshape(t_h, 128)
            abs_base = bounds[c] + base_n
            cnt1[:width, g] = 1.0 + deg[abs_base : abs_base + width]
            fself[g * 128 : g * 128 + width] = feat[abs_base : abs_base + width]

        gidx_w = np.ascontiguousarray(np.tile(gidx_all.reshape(-1, 16).T, (8, 1)))
        srel_mat = np.ascontiguousarray(srel_all.T)
        in_maps.append(
            {
                "features": feat,
                "gidx": gidx_w,
                "srel": srel_mat,
                "cnt1": cnt1,
                "fself": fself,
            }
        )
        slot_maps.append(
            [(bounds[c] + sl[0], sl[1]) for sl in slots]
        )
    return t_l_arr, t_h_arr, in_maps, slot_maps


def kernel(features, neighbor_idx, segment_ids):
    global LAST_NC
    t_l_arr, t_h_arr, in_maps, slot_maps = _preprocess(
        features, neighbor_idx, segment_ids
    )

    key = (t_l_arr, t_h_arr)
    if key not in _PROGRAM_CACHE:
        _PROGRAM_CACHE[key] = _build_program(t_l_arr, t_h_arr)
    nc = _PROGRAM_CACHE[key]
    LAST_NC = nc

    res = bass_utils.run_bass_kernel_spmd(
        nc, in_maps, core_ids=list(range(N_CORES))
    )

    out = np.empty((N_NODES, D), np.float32)
    for c in range(N_CORES):
        oc = res.results[c]["out"]
        for g, (abs_base, width) in enumerate(slot_maps[c]):
            out[abs_base : abs_base + width] = oc[g * 128 : g * 128 + width]
    return out
